# revision 11
# baseline (speedup 1.0000x reference)
"""GatedGraphConv (single-step GGNN) Trainium2 Bass kernel, 8-core SPMD.

Strategy (dst-sharded, gather-based):
- Shard destination nodes across 8 cores (12500 nodes / 50000 (node,type)
  segments per core). Each core processes the ~125k edges pointing at its
  nodes; the node-feature table is replicated in each core's DRAM.
- Edge pipeline per core, organized in 4 "bank passes" (source-node banks
  of 25000 rows so dma_gather's int16 indices reach every row), edges
  seg-sorted within a pass and grouped into chunks of 128 edges whose
  segment span is < 128:
    dma_gather (SWDGE, 256B bf16 rows)  ->  msgs [128e, 128] tiles
    tensor_scalar (DVE): S = (iota == segoff)*w  [128e, 128seg] one-hot
    matmul (PE): psum[64f, 128seg] = msgs^T @ S   (chunk-local, start/stop)
    tensor_add (DVE): update^T[:, segbase:segbase+128] += psum
      (segbase loaded from a per-core table into a register -> dynamic AP
       offset, so one SPMD program serves 8 different edge distributions)
- update^T [64, 50176] bf16 stays in SBUF; MLP (relu(W@u+b)) and the GRU
  cell run on PE/ACT/DVE in feature-major layout; a final PE transpose
  emits row-major fp32 output.
"""

import sys
import types

sys.path.insert(0, "/opt/trn_rl_repo")
sys.path.insert(0, "/root/.axon_site")

import numpy as np
import ml_dtypes

import concourse.bass as bass
import concourse.bacc as bacc
from concourse import tile, mybir
from concourse.bass_utils import run_bass_kernel_spmd

BF16 = ml_dtypes.bfloat16

# ---------------------------------------------------------------- dims

N_CORES = 8
T_TYPES = 4
D = 64            # feature dim
DP = 128          # padded row elems (bf16) -> 256B gather rows
H = 256           # mlp hidden
G3 = 192          # 3 * D gru gates

REAL = dict(
    n_nodes=100000,
    nloc=12500,       # dst nodes per core
    bank=25000,       # src rows per gather bank (int16 index limit)
    chunks_per_gather=16,   # 2048 idxs per dma_gather instruction
)


def _register_ntff_hook():
    """The image's antenv lacks axon_hooks; register the NTFF profile hook
    so trace=True yields exec_time_ns."""
    if "antenv.axon_hooks" in sys.modules:
        return
    try:
        import trn_agent_boot.trn_boot as tb
        hook = tb._ntff_profile_via_ctypes("/opt/axon/libaxon_pjrt.so")
        mod = types.ModuleType("antenv.axon_hooks")
        mod.get_axon_ntff_profile_hook = lambda: hook
        sys.modules["antenv.axon_hooks"] = mod
    except Exception:
        pass


# ---------------------------------------------------------------- host prep

def _chunk_core_edges(src, seg, w, bank, n_banks):
    """Split one core's edges into per-bank chunk lists.
    Returns per bank: list of chunks, each (idx[128] int16 local-bank row,
    segoff[128] f32, w[128] f32, segbase int32)."""
    out = []
    for b in range(n_banks):
        m = (src // bank) == b
        s, g, ww = src[m] % bank, seg[m], w[m]
        o = np.argsort(g, kind="stable")
        s, g, ww = s[o], g[o], ww[o]
        chunks = []
        i, n = 0, len(g)
        while i < n:
            base = g[i]
            j = min(i + 128, n)
            # shrink so the chunk's seg span stays < 128
            hi = np.searchsorted(g[i:j], base + 128, side="left")
            j = i + hi
            k = j - i
            idx = np.zeros(128, np.int16)
            off = np.zeros(128, np.float32)
            wgt = np.zeros(128, np.float32)
            idx[:k] = s[i:j]
            off[:k] = (g[i:j] - base).astype(np.float32)
            wgt[:k] = ww[i:j]
            chunks.append((idx, off, wgt, base))
            i = j
        out.append(chunks)
    return out


def _pad_chunks(per_core_banks, n_banks, cpg):
    """Equalize chunk counts per bank across cores (SPMD uniformity) and
    round to the gather-instruction granularity."""
    ncs = []
    for b in range(n_banks):
        mx = max(len(c[b]) for c in per_core_banks)
        mx = ((mx + cpg - 1) // cpg) * cpg
        ncs.append(max(mx, cpg))
    pad = (np.zeros(128, np.int16), np.zeros(128, np.float32),
           np.zeros(128, np.float32), 0)
    for c in per_core_banks:
        for b in range(n_banks):
            c[b].extend([pad] * (ncs[b] - len(c[b])))
    return ncs


def _wrap_idx(idx_flat):
    """gather idx layout: position i -> (partition i%16, col i//16),
    replicated across the 8 Q7 cores -> [128, n/16]."""
    n = idx_flat.shape[0]
    a = idx_flat.reshape(n // 16, 16).T
    return np.ascontiguousarray(np.tile(a, (8, 1)))


def _host_prep(node_feature, edge_index, edge_type, edge_weight, dims):
    nloc, bank = dims["nloc"], dims["bank"]
    n_nodes = dims["n_nodes"]
    n_banks = (n_nodes + bank - 1) // bank
    cpg = dims["chunks_per_gather"]

    src = np.asarray(edge_index[0], np.int64)
    dst = np.asarray(edge_index[1], np.int64)
    et = np.asarray(edge_type, np.int64)
    w = np.asarray(edge_weight, np.float32)

    core = dst // nloc
    per_core = []
    for c in range(N_CORES):
        m = core == c
        seg = et[m] * nloc + (dst[m] - c * nloc)   # t-major local segment id
        per_core.append(_chunk_core_edges(src[m].astype(np.int64), seg, w[m],
                                          bank, n_banks))
    ncs = _pad_chunks(per_core, n_banks, cpg)

    # flatten to per-core arrays
    segs_pad = ((T_TYPES * nloc + 127) // 128 + 1) * 128
    per_core_arrays = []
    for c in range(N_CORES):
        idxs, offs, wgts, bases = [], [], [], []
        for b in range(n_banks):
            for (idx, off, wgt, base) in per_core[c][b]:
                idxs.append(idx)
                offs.append(off)
                wgts.append(wgt)
                bases.append(base)
        nch = len(bases)
        idx_flat = np.concatenate(idxs)                    # [nch*128]
        gidx = np.concatenate(
            [_wrap_idx(idx_flat[g * cpg * 128:(g + 1) * cpg * 128])
             for g in range(nch // cpg)], axis=1)          # [128, nch*8]
        segoff = np.stack(offs, axis=1)                    # [128, nch]
        wcol = np.stack(wgts, axis=1)                      # [128, nch]
        segbase = np.asarray(bases, np.int32)[None, :]     # [1, nch]
        per_core_arrays.append(dict(gidx=gidx, soff=segoff, wcol=wcol,
                                    sbase=segbase))

    # node table, bf16, rows padded to 128 elems (256B)
    tbl = np.zeros((n_nodes, DP), dtype=BF16)
    tbl[:, :D] = node_feature.astype(BF16)

    meta = dict(ncs=ncs, n_banks=n_banks, segs_pad=segs_pad)
    return per_core_arrays, tbl, meta


def _prep_weights(mlp_W, mlp_b, w_ih, w_hh, b_ih, b_hh, nloc):
    """Blocked, transposed weight layouts (identical on every core)."""
    out = {}
    # MLP lhsT tiles [64 f, 128 h] for (htile k, type t): col index k*4+t
    mw = np.zeros((D, 8, 128), dtype=BF16)
    for k in range(2):
        for t in range(T_TYPES):
            mw[:, k * 4 + t, :] = mlp_W[128 * k:128 * (k + 1),
                                        D * t:D * (t + 1)].T.astype(BF16)
    out["mlpw"] = mw.reshape(D, 8 * 128)
    out["mlpb"] = mlp_b.reshape(2, 128).T.astype(np.float32)  # [128, 2]
    # GRU gi lhsT: [128 h(p), 192] per contraction chunk hc
    wi = np.zeros((128, 2, G3), dtype=BF16)
    for hc in range(2):
        wi[:, hc, :] = w_ih[:, 128 * hc:128 * (hc + 1)].T.astype(BF16)
    out["wih"] = wi.reshape(128, 2 * G3)
    out["whh"] = w_hh.T.astype(BF16)                       # [64, 192]
    gb = (b_ih + b_hh).astype(np.float32)
    out["b_r"] = gb[:D].reshape(D, 1)
    out["b_z"] = gb[D:2 * D].reshape(D, 1)
    # n-gate: keep b_in and b_hn separate (n = tanh(i_n+b_in + r*(h_n+b_hn)))
    out["b_in"] = b_ih[128:].astype(np.float32).reshape(D, 1)
    out["b_hn"] = b_hh[128:].astype(np.float32).reshape(D, 1)
    out["iota"] = np.tile(np.arange(128, dtype=np.float32), (128, 1)).astype(BF16)
    out["ident"] = np.eye(128, dtype=np.float32)
    return out


# ---------------------------------------------------------------- program

def _build_program(dims, meta):
    nloc = dims["nloc"]
    bank = dims["bank"]
    cpg = dims["chunks_per_gather"]
    ncs = meta["ncs"]
    n_banks = meta["n_banks"]
    segs_pad = meta["segs_pad"]
    nch = sum(ncs)
    n_nodes = dims["n_nodes"]
    ntp = (nloc + 127) // 128 * 128          # padded node count (rows out)
    NT = 512                                  # node-tile width for mlp/gru
    n_nt = (nloc + NT - 1) // NT

    nc = bacc.Bacc("TRN2", target_bir_lowering=False, debug=False,
                   num_devices=N_CORES, dynamic_dma_scratch_size=32768)

    f32, bf16, i16, i32 = (mybir.dt.float32, mybir.dt.bfloat16,
                           mybir.dt.int16, mybir.dt.int32)

    t_tbl = nc.dram_tensor("tbl", [n_nodes, DP], bf16, kind="ExternalInput")
    t_gidx = nc.dram_tensor("gidx", [128, nch * 8], i16, kind="ExternalInput")
    t_soff = nc.dram_tensor("soff", [128, nch], f32, kind="ExternalInput")
    t_wcol = nc.dram_tensor("wcol", [128, nch], f32, kind="ExternalInput")
    t_sbase = nc.dram_tensor("sbase", [1, nch], i32, kind="ExternalInput")
    t_xtb = nc.dram_tensor("xtb", [D, ntp], bf16, kind="ExternalInput")
    t_xtf = nc.dram_tensor("xtf", [D, ntp], f32, kind="ExternalInput")
    t_mlpw = nc.dram_tensor("mlpw", [D, 8 * 128], bf16, kind="ExternalInput")
    t_mlpb = nc.dram_tensor("mlpb", [128, 2], f32, kind="ExternalInput")
    t_wih = nc.dram_tensor("wih", [128, 2 * G3], bf16, kind="ExternalInput")
    t_whh = nc.dram_tensor("whh", [D, G3], bf16, kind="ExternalInput")
    t_br = nc.dram_tensor("br", [D, 1], f32, kind="ExternalInput")
    t_bz = nc.dram_tensor("bz", [D, 1], f32, kind="ExternalInput")
    t_bin = nc.dram_tensor("bin", [D, 1], f32, kind="ExternalInput")
    t_bhn = nc.dram_tensor("bhn", [D, 1], f32, kind="ExternalInput")
    t_iota = nc.dram_tensor("iota", [128, 128], bf16, kind="ExternalInput")
    t_ident = nc.dram_tensor("ident", [128, 128], f32, kind="ExternalInput")
    t_out = nc.dram_tensor("out", [ntp, D], f32, kind="ExternalOutput")

    with tile.TileContext(nc) as tc:
        with tc.tile_pool(name="const", bufs=1) as cp:
            soff_t = cp.tile([128, nch], f32)
            nc.sync.dma_start(out=soff_t[:], in_=t_soff[:])
            wcol_t = cp.tile([128, nch], f32)
            nc.sync.dma_start(out=wcol_t[:], in_=t_wcol[:])
            sbase_t = cp.tile([1, nch], i32)
            nc.sync.dma_start(out=sbase_t[:], in_=t_sbase[:])
            iota_t = cp.tile([128, 128], bf16)
            nc.sync.dma_start(out=iota_t[:], in_=t_iota[:])

            upd = cp.tile([D, segs_pad], bf16)
            nc.vector.memset(upd[:], 0.0)
            off_reg = nc.vector.alloc_register("segoff_dyn")

            # ---------------- phase 1: gather + segment scatter -------
            with tc.tile_pool(name="gt", bufs=4) as gtp, \
                 tc.tile_pool(name="gi", bufs=2) as gip, \
                 tc.tile_pool(name="sp", bufs=8) as spool, \
                 tc.tile_pool(name="ps", bufs=8, space="PSUM") as psp:
                cbase = 0
                for b in range(n_banks):
                    tbl_b = t_tbl[b * bank:min((b + 1) * bank, n_nodes), :]
                    gixw = max(ncs) * 8
                    gidx_t = gip.tile([128, gixw], i16, tag="gix")
                    nc.sync.dma_start(
                        out=gidx_t[:, :ncs[b] * 8],
                        in_=t_gidx[:, cbase * 8:(cbase + ncs[b]) * 8])
                    for g in range(ncs[b] // cpg):
                        gt = gtp.tile([128, cpg, DP], bf16, tag="g")
                        c0 = cbase + g * cpg
                        gl = g * cpg
                        nc.gpsimd.dma_gather(
                            gt[:], tbl_b,
                            gidx_t[:, gl * 8:(gl + cpg) * 8],
                            cpg * 128, cpg * 128, DP,
                            single_packet=False,
                        )
                        for cl in range(cpg):
                            c = c0 + cl
                            s_t = spool.tile([128, 128], bf16, tag="S")
                            nc.vector.tensor_scalar(
                                s_t[:], iota_t[:],
                                soff_t[:, c:c + 1], wcol_t[:, c:c + 1],
                                mybir.AluOpType.is_equal, mybir.AluOpType.mult,
                            )
                            pm = psp.tile([D, 128], f32, tag="pm")
                            nc.tensor.matmul(
                                out=pm[:], lhsT=gt[:, cl, 0:D], rhs=s_t[:],
                                start=True, stop=True,
                            )
                            nc.vector.reg_load(off_reg, sbase_t[0:1, c:c + 1])
                            off = nc.vector.snap(
                                off_reg, min_val=0, max_val=segs_pad - 128)
                            dst = upd[:, bass.ds(off, 128)]
                            nc.vector.tensor_add(out=dst, in0=dst, in1=pm[:])
                    cbase += ncs[b]

            # ---------------- phase 2+3: MLP + GRU + transpose --------
            mlpw_t = cp.tile([D, 8 * 128], bf16)
            nc.sync.dma_start(out=mlpw_t[:], in_=t_mlpw[:])
            mlpb_t = cp.tile([128, 2], f32)
            nc.sync.dma_start(out=mlpb_t[:], in_=t_mlpb[:])
            wih_t = cp.tile([128, 2 * G3], bf16)
            nc.sync.dma_start(out=wih_t[:], in_=t_wih[:])
            whh_t = cp.tile([D, G3], bf16)
            nc.sync.dma_start(out=whh_t[:], in_=t_whh[:])
            br_t = cp.tile([D, 1], f32)
            nc.sync.dma_start(out=br_t[:], in_=t_br[:])
            bz_t = cp.tile([D, 1], f32)
            nc.sync.dma_start(out=bz_t[:], in_=t_bz[:])
            bin_t = cp.tile([D, 1], f32)
            nc.sync.dma_start(out=bin_t[:], in_=t_bin[:])
            bhn_t = cp.tile([D, 1], f32)
            nc.sync.dma_start(out=bhn_t[:], in_=t_bhn[:])
            ident_t = cp.tile([128, 128], f32)
            nc.sync.dma_start(out=ident_t[:], in_=t_ident[:])

            with tc.tile_pool(name="mm", bufs=2) as mp, \
                 tc.tile_pool(name="pp", bufs=1, space="PSUM") as pp2, \
                 tc.tile_pool(name="pt", bufs=2, space="PSUM") as ppt:
                for it in range(n_nt):
                    lo = it * NT
                    hi = min(lo + NT, nloc)
                    n = hi - lo
                    xb = mp.tile([D, NT], bf16, tag="xb")
                    nc.sync.dma_start(out=xb[:, :n], in_=t_xtb[:, lo:hi])
                    xf = mp.tile([D, NT], f32, tag="xf")
                    nc.sync.dma_start(out=xf[:, :n], in_=t_xtf[:, lo:hi])
                    hid = []
                    for k in range(2):
                        hk = mp.tile([128, NT], bf16, tag=f"hid{k}")
                        hid.append(hk)
                    # ---- MLP: hidden[k] = relu(sum_t Wt @ upd_t + b)
                    for k in range(2):
                        ph = pp2.tile([128, NT], f32, tag="ph")
                        for t in range(T_TYPES):
                            nc.tensor.matmul(
                                out=ph[:, :n],
                                lhsT=mlpw_t[:, (k * 4 + t) * 128:(k * 4 + t + 1) * 128],
                                rhs=upd[:, t * nloc + lo:t * nloc + hi],
                                start=(t == 0), stop=(t == 3),
                            )
                        nc.scalar.activation(
                            hid[k][:, :n], ph[:, :n],
                            mybir.ActivationFunctionType.Relu,
                            bias=mlpb_t[:, k:k + 1], scale=1.0,
                        )
                    # ---- GRU gates
                    # r and z gates, each [64, n] on partitions 0:63
                    gate_sb = []
                    for gi_, bias_t in ((0, br_t), (1, bz_t)):
                        pg = pp2.tile([D, NT], f32, tag=f"pg{gi_}")
                        for hc in range(2):
                            nc.tensor.matmul(
                                out=pg[:, :n],
                                lhsT=wih_t[:, hc * G3 + gi_ * D:hc * G3 + (gi_ + 1) * D],
                                rhs=hid[hc][:, :n],
                                start=(hc == 0), stop=False,
                            )
                        nc.tensor.matmul(
                            out=pg[:, :n], lhsT=whh_t[:, gi_ * D:(gi_ + 1) * D],
                            rhs=xb[:, :n], start=False, stop=True,
                        )
                        gsb = mp.tile([D, NT], f32, tag=f"g{gi_}")
                        nc.scalar.activation(
                            gsb[:, :n], pg[:, :n],
                            mybir.ActivationFunctionType.Sigmoid,
                            bias=bias_t[:], scale=1.0,
                        )
                        gate_sb.append(gsb)
                    r_sb, z_sb = gate_sb
                    # i_n psum [64, n]
                    pin = pp2.tile([D, NT], f32, tag="pin")
                    for hc in range(2):
                        nc.tensor.matmul(
                            out=pin[:, :n],
                            lhsT=wih_t[:, hc * G3 + 128:hc * G3 + G3],
                            rhs=hid[hc][:, :n],
                            start=(hc == 0), stop=(hc == 1),
                        )
                    # h_n psum [64, n]
                    phn = pp2.tile([D, NT], f32, tag="phn")
                    nc.tensor.matmul(
                        out=phn[:, :n], lhsT=whh_t[:, 128:G3],
                        rhs=xb[:, :n], start=True, stop=True,
                    )
                    hn = mp.tile([D, NT], f32, tag="hn")
                    nc.scalar.activation(
                        hn[:, :n], phn[:, :n],
                        mybir.ActivationFunctionType.Identity,
                        bias=bhn_t[:], scale=1.0,
                    )
                    t1 = mp.tile([D, NT], f32, tag="t1")
                    nc.vector.tensor_mul(t1[:, :n], r_sb[:, :n], hn[:, :n])
                    # t2 = (pin + b_in) + t1
                    t2 = mp.tile([D, NT], f32, tag="t2")
                    nc.vector.scalar_tensor_tensor(
                        t2[:, :n], pin[:, :n], bin_t[:], t1[:, :n],
                        mybir.AluOpType.add, mybir.AluOpType.add,
                    )
                    ng = mp.tile([D, NT], f32, tag="ng")
                    nc.scalar.activation(
                        ng[:, :n], t2[:, :n],
                        mybir.ActivationFunctionType.Tanh,
                        bias=0.0, scale=1.0,
                    )
                    # out = n + z*(x - n)
                    t3 = mp.tile([D, NT], f32, tag="t3")
                    nc.vector.tensor_sub(t3[:, :n], xf[:, :n], ng[:, :n])
                    t4 = mp.tile([D, NT], f32, tag="t4")
                    nc.vector.tensor_mul(t4[:, :n], z_sb[:, :n], t3[:, :n])
                    ot = mp.tile([D, NT], f32, tag="ot")
                    nc.vector.tensor_add(ot[:, :n], ng[:, :n], t4[:, :n])
                    # ---- transpose to rows and store
                    for q in range(0, NT, 128):
                        if lo + q >= nloc:
                            break
                        qn = min(128, nloc - lo - q)
                        pt = ppt.tile([128, D], f32, tag="pt")
                        nc.tensor.transpose(
                            out=pt[:], in_=ot[:, q:q + 128],
                            identity=ident_t[0:D, 0:D],
                        )
                        rows = mp.tile([128, D], f32, tag="rows")
                        nc.vector.tensor_copy(rows[:], pt[:])
                        nc.sync.dma_start(
                            out=t_out[lo + q:lo + q + qn, :],
                            in_=rows[:qn, :])

    nc.compile()
    return nc


# ---------------------------------------------------------------- entry

_CACHE = {}


def _build_in_maps(inputs, dims):
    node_feature = np.asarray(inputs["node_feature"], np.float32)
    per_core_arrays, tbl, meta = _host_prep(
        node_feature, np.asarray(inputs["edge_index"]),
        np.asarray(inputs["edge_type"]),
        np.asarray(inputs["edge_weight"], np.float32), dims)
    wts = _prep_weights(
        np.asarray(inputs["mlp_W"], np.float32),
        np.asarray(inputs["mlp_b"], np.float32),
        np.asarray(inputs["w_ih"], np.float32),
        np.asarray(inputs["w_hh"], np.float32),
        np.asarray(inputs["b_ih"], np.float32),
        np.asarray(inputs["b_hh"], np.float32), dims["nloc"])

    nloc = dims["nloc"]
    ntp = (nloc + 127) // 128 * 128
    in_maps = []
    for c in range(N_CORES):
        x_own = node_feature[c * nloc:(c + 1) * nloc]       # [nloc, 64]
        xt = np.zeros((D, ntp), np.float32)
        xt[:, :nloc] = x_own.T
        m = dict(per_core_arrays[c])
        m.update(
            tbl=tbl,
            xtb=xt.astype(BF16), xtf=xt,
            mlpw=wts["mlpw"], mlpb=wts["mlpb"], wih=wts["wih"],
            whh=wts["whh"], br=wts["b_r"], bz=wts["b_z"], bin=wts["b_in"],
            bhn=wts["b_hn"], iota=wts["iota"], ident=wts["ident"],
        )
        in_maps.append(m)
    return in_maps, meta


def _run(inputs, trace=False):
    _register_ntff_hook()
    dims = dict(REAL)
    in_maps, meta = _build_in_maps(inputs, dims)
    key = ("real", tuple(meta["ncs"]))
    if key not in _CACHE:
        _CACHE[key] = _build_program(dims, meta)
    nc = _CACHE[key]
    res = run_bass_kernel_spmd(nc, in_maps, list(range(N_CORES)), trace=trace)
    nloc = dims["nloc"]
    out = np.concatenate(
        [res.results[c]["out"][:nloc] for c in range(N_CORES)], axis=0)
    return out.astype(np.float32), res


def kernel(**inputs) -> np.ndarray:
    return _run(inputs, trace=False)[0]


# revision 13
# speedup vs baseline: 1.9284x; 1.9284x over previous
"""GatedGraphConv (single-step GGNN) Trainium2 Bass kernel, 8-core SPMD.

Strategy (dst-sharded, gather-based):
- Shard destination nodes across 8 cores (12500 nodes / 50000 (node,type)
  segments per core). Each core processes the ~125k edges pointing at its
  nodes; the node-feature table is replicated in each core's DRAM.
- Edge pipeline per core, organized in 4 "bank passes" (source-node banks
  of 25000 rows so dma_gather's int16 indices reach every row), edges
  seg-sorted within a pass and grouped into chunks of 128 edges whose
  segment span is < 128:
    dma_gather (SWDGE, 256B bf16 rows)  ->  msgs [128e, 128] tiles
    tensor_scalar (DVE): S = (iota == segoff)*w  [128e, 128seg] one-hot
    matmul (PE): psum[64f, 128seg] = msgs^T @ S   (chunk-local, start/stop)
    tensor_add (DVE): update^T[:, segbase:segbase+128] += psum
      (segbase loaded from a per-core table into a register -> dynamic AP
       offset, so one SPMD program serves 8 different edge distributions)
- update^T [64, 50176] bf16 stays in SBUF; MLP (relu(W@u+b)) and the GRU
  cell run on PE/ACT/DVE in feature-major layout; a final PE transpose
  emits row-major fp32 output.
"""

import sys
import types

sys.path.insert(0, "/opt/trn_rl_repo")
sys.path.insert(0, "/root/.axon_site")

import numpy as np
import ml_dtypes

import concourse.bass as bass
import concourse.bacc as bacc
from concourse import tile, mybir
from concourse.bass_utils import run_bass_kernel_spmd

BF16 = ml_dtypes.bfloat16

# ---------------------------------------------------------------- dims

N_CORES = 8
T_TYPES = 4
D = 64            # feature dim
DP = 128          # padded row elems (bf16) -> 256B gather rows
H = 256           # mlp hidden
G3 = 192          # 3 * D gru gates

REAL = dict(
    n_nodes=100000,
    nloc=12500,       # dst nodes per core
    bank=25000,       # src rows per gather bank (int16 index limit)
    chunks_per_gather=16,   # 2048 idxs per dma_gather instruction
)


def _register_ntff_hook():
    """The image's antenv lacks axon_hooks; register the NTFF profile hook
    so trace=True yields exec_time_ns."""
    if "antenv.axon_hooks" in sys.modules:
        return
    try:
        import trn_agent_boot.trn_boot as tb
        hook = tb._ntff_profile_via_ctypes("/opt/axon/libaxon_pjrt.so")
        mod = types.ModuleType("antenv.axon_hooks")
        mod.get_axon_ntff_profile_hook = lambda: hook
        sys.modules["antenv.axon_hooks"] = mod
    except Exception:
        pass


# ---------------------------------------------------------------- host prep

SW = 256  # segment window width per chunk (S matrix columns)


def _chunk_core_edges(src, seg, w, bank, n_banks):
    """Split one core's edges into per-bank chunk lists.
    Returns per bank: list of chunks, each (idx[128] int16 local-bank row,
    segoff[128] f32, w[128] f32, segbase int32)."""
    out = []
    for b in range(n_banks):
        m = (src // bank) == b
        s, g, ww = src[m] % bank, seg[m], w[m]
        o = np.argsort(g, kind="stable")
        s, g, ww = s[o], g[o], ww[o]
        chunks = []
        i, n = 0, len(g)
        while i < n:
            base = g[i]
            j = min(i + 128, n)
            # shrink so the chunk's seg span stays < 128
            hi = np.searchsorted(g[i:j], base + SW, side="left")
            j = i + hi
            k = j - i
            idx = np.zeros(128, np.int16)
            off = np.zeros(128, np.float32)
            wgt = np.zeros(128, np.float32)
            idx[:k] = s[i:j]
            off[:k] = (g[i:j] - base).astype(np.float32)
            wgt[:k] = ww[i:j]
            chunks.append((idx, off, wgt, base))
            i = j
        out.append(chunks)
    return out


def _pad_chunks(per_core_banks, n_banks, cpg):
    """Equalize chunk counts per bank across cores (SPMD uniformity) and
    round to the gather-instruction granularity."""
    ncs = []
    for b in range(n_banks):
        mx = max(len(c[b]) for c in per_core_banks)
        mx = ((mx + cpg - 1) // cpg) * cpg
        ncs.append(max(mx, cpg))
    pad = (np.zeros(128, np.int16), np.zeros(128, np.float32),
           np.zeros(128, np.float32), 0)
    for c in per_core_banks:
        for b in range(n_banks):
            c[b].extend([pad] * (ncs[b] - len(c[b])))
    return ncs


def _wrap_idx(idx_flat):
    """gather idx layout: position i -> (partition i%16, col i//16),
    replicated across the 8 Q7 cores -> [128, n/16]."""
    n = idx_flat.shape[0]
    a = idx_flat.reshape(n // 16, 16).T
    return np.ascontiguousarray(np.tile(a, (8, 1)))


def _host_prep(node_feature, edge_index, edge_type, edge_weight, dims):
    nloc, bank = dims["nloc"], dims["bank"]
    n_nodes = dims["n_nodes"]
    n_banks = (n_nodes + bank - 1) // bank
    cpg = dims["chunks_per_gather"]

    src = np.asarray(edge_index[0], np.int64)
    dst = np.asarray(edge_index[1], np.int64)
    et = np.asarray(edge_type, np.int64)
    w = np.asarray(edge_weight, np.float32)

    core = dst // nloc
    per_core = []
    for c in range(N_CORES):
        m = core == c
        seg = et[m] * nloc + (dst[m] - c * nloc)   # t-major local segment id
        per_core.append(_chunk_core_edges(src[m].astype(np.int64), seg, w[m],
                                          bank, n_banks))
    ncs = _pad_chunks(per_core, n_banks, cpg)

    # flatten to per-core arrays
    segs_pad = ((T_TYPES * nloc + SW + 127) // 128) * 128
    per_core_arrays = []
    for c in range(N_CORES):
        idxs, offs, wgts, bases = [], [], [], []
        for b in range(n_banks):
            for (idx, off, wgt, base) in per_core[c][b]:
                idxs.append(idx)
                offs.append(off)
                wgts.append(wgt)
                bases.append(base)
        nch = len(bases)
        idx_flat = np.concatenate(idxs)                    # [nch*128]
        gidx = np.concatenate(
            [_wrap_idx(idx_flat[g * cpg * 128:(g + 1) * cpg * 128])
             for g in range(nch // cpg)], axis=1)          # [128, nch*8]
        segoff = np.stack(offs, axis=1)                    # [128, nch]
        wcol = np.stack(wgts, axis=1)                      # [128, nch]
        segbase = np.asarray(bases, np.int32)[None, :]     # [1, nch]
        # host-built one-hot scatter matrices, streamed to the PE:
        # sst[p, c*SW + segoff[p,c]] = w[p,c]
        sst = np.zeros((128, nch * SW), dtype=BF16)
        pp, cc = np.meshgrid(np.arange(128), np.arange(nch), indexing="ij")
        sst[pp.ravel(), (cc * SW + segoff.astype(np.int64)).ravel()] = \
            wcol.ravel().astype(BF16)
        per_core_arrays.append(dict(gidx=gidx, sst=sst, sbase=segbase))

    # node table, bf16, rows padded to 128 elems (256B)
    tbl = np.zeros((n_nodes, DP), dtype=BF16)
    tbl[:, :D] = node_feature.astype(BF16)

    meta = dict(ncs=ncs, n_banks=n_banks, segs_pad=segs_pad)
    return per_core_arrays, tbl, meta


def _prep_weights(mlp_W, mlp_b, w_ih, w_hh, b_ih, b_hh, nloc):
    """Blocked, transposed weight layouts (identical on every core)."""
    out = {}
    # MLP lhsT tiles [64 f, 128 h] for (htile k, type t): col index k*4+t
    mw = np.zeros((D, 8, 128), dtype=BF16)
    for k in range(2):
        for t in range(T_TYPES):
            mw[:, k * 4 + t, :] = mlp_W[128 * k:128 * (k + 1),
                                        D * t:D * (t + 1)].T.astype(BF16)
    out["mlpw"] = mw.reshape(D, 8 * 128)
    out["mlpb"] = mlp_b.reshape(2, 128).T.astype(np.float32)  # [128, 2]
    # GRU gi lhsT: [128 h(p), 192] per contraction chunk hc
    wi = np.zeros((128, 2, G3), dtype=BF16)
    for hc in range(2):
        wi[:, hc, :] = w_ih[:, 128 * hc:128 * (hc + 1)].T.astype(BF16)
    out["wih"] = wi.reshape(128, 2 * G3)
    out["whh"] = w_hh.T.astype(BF16)                       # [64, 192]
    gb = (b_ih + b_hh).astype(np.float32)
    out["b_r"] = gb[:D].reshape(D, 1)
    out["b_z"] = gb[D:2 * D].reshape(D, 1)
    # n-gate: keep b_in and b_hn separate (n = tanh(i_n+b_in + r*(h_n+b_hn)))
    out["b_in"] = b_ih[128:].astype(np.float32).reshape(D, 1)
    out["b_hn"] = b_hh[128:].astype(np.float32).reshape(D, 1)
    out["ident"] = np.eye(128, dtype=np.float32)
    return out


# ---------------------------------------------------------------- program

def _build_program(dims, meta):
    nloc = dims["nloc"]
    bank = dims["bank"]
    cpg = dims["chunks_per_gather"]
    ncs = meta["ncs"]
    n_banks = meta["n_banks"]
    segs_pad = meta["segs_pad"]
    nch = sum(ncs)
    n_nodes = dims["n_nodes"]
    ntp = (nloc + 127) // 128 * 128          # padded node count (rows out)
    NT = 512                                  # node-tile width for mlp/gru
    n_nt = (nloc + NT - 1) // NT

    nc = bacc.Bacc("TRN2", target_bir_lowering=False, debug=False,
                   num_devices=N_CORES, dynamic_dma_scratch_size=32768)

    f32, bf16, i16, i32 = (mybir.dt.float32, mybir.dt.bfloat16,
                           mybir.dt.int16, mybir.dt.int32)

    t_tbl = nc.dram_tensor("tbl", [n_nodes, DP], bf16, kind="ExternalInput")
    t_gidx = nc.dram_tensor("gidx", [128, nch * 8], i16, kind="ExternalInput")
    t_sst = nc.dram_tensor("sst", [128, nch * SW], bf16, kind="ExternalInput")
    t_sbase = nc.dram_tensor("sbase", [1, nch], i32, kind="ExternalInput")
    t_xtb = nc.dram_tensor("xtb", [D, ntp], bf16, kind="ExternalInput")
    t_xtf = nc.dram_tensor("xtf", [D, ntp], f32, kind="ExternalInput")
    t_mlpw = nc.dram_tensor("mlpw", [D, 8 * 128], bf16, kind="ExternalInput")
    t_mlpb = nc.dram_tensor("mlpb", [128, 2], f32, kind="ExternalInput")
    t_wih = nc.dram_tensor("wih", [128, 2 * G3], bf16, kind="ExternalInput")
    t_whh = nc.dram_tensor("whh", [D, G3], bf16, kind="ExternalInput")
    t_br = nc.dram_tensor("br", [D, 1], f32, kind="ExternalInput")
    t_bz = nc.dram_tensor("bz", [D, 1], f32, kind="ExternalInput")
    t_bin = nc.dram_tensor("bin", [D, 1], f32, kind="ExternalInput")
    t_bhn = nc.dram_tensor("bhn", [D, 1], f32, kind="ExternalInput")
    t_ident = nc.dram_tensor("ident", [128, 128], f32, kind="ExternalInput")
    t_out = nc.dram_tensor("out", [ntp, D], f32, kind="ExternalOutput")

    with tile.TileContext(nc) as tc:
        with tc.tile_pool(name="const", bufs=1) as cp:
            sbase_t = cp.tile([1, nch], i32)
            nc.sync.dma_start(out=sbase_t[:], in_=t_sbase[:])

            upd = cp.tile([D, segs_pad], bf16)
            nc.vector.memset(upd[:], 0.0)
            off_reg = nc.vector.alloc_register("segoff_dyn")

            # ---------------- phase 1: gather + segment scatter -------
            with tc.tile_pool(name="gt", bufs=4) as gtp, \
                 tc.tile_pool(name="gi", bufs=2) as gip, \
                 tc.tile_pool(name="sp", bufs=2) as spool, \
                 tc.tile_pool(name="ps", bufs=8, space="PSUM") as psp:
                cbase = 0
                for b in range(n_banks):
                    tbl_b = t_tbl[b * bank:min((b + 1) * bank, n_nodes), :]
                    gixw = max(ncs) * 8
                    gidx_t = gip.tile([128, gixw], i16, tag="gix")
                    nc.sync.dma_start(
                        out=gidx_t[:, :ncs[b] * 8],
                        in_=t_gidx[:, cbase * 8:(cbase + ncs[b]) * 8])
                    for g in range(ncs[b] // cpg):
                        gt = gtp.tile([128, cpg, DP], bf16, tag="g")
                        c0 = cbase + g * cpg
                        gl = g * cpg
                        nc.gpsimd.dma_gather(
                            gt[:], tbl_b,
                            gidx_t[:, gl * 8:(gl + cpg) * 8],
                            cpg * 128, cpg * 128, DP,
                            single_packet=False,
                        )
                        s_t = spool.tile([128, cpg * SW], bf16, tag="S")
                        nc.sync.dma_start(
                            out=s_t[:],
                            in_=t_sst[:, c0 * SW:(c0 + cpg) * SW])
                        for cl in range(cpg):
                            c = c0 + cl
                            pm = psp.tile([D, SW], f32, tag="pm")
                            nc.tensor.matmul(
                                out=pm[:], lhsT=gt[:, cl, 0:D],
                                rhs=s_t[:, cl * SW:(cl + 1) * SW],
                                start=True, stop=True,
                            )
                            nc.vector.reg_load(off_reg, sbase_t[0:1, c:c + 1])
                            off = nc.vector.snap(
                                off_reg, min_val=0, max_val=segs_pad - SW)
                            dst = upd[:, bass.ds(off, SW)]
                            nc.vector.tensor_add(out=dst, in0=dst, in1=pm[:])
                    cbase += ncs[b]

            # ---------------- phase 2+3: MLP + GRU + transpose --------
            mlpw_t = cp.tile([D, 8 * 128], bf16)
            nc.sync.dma_start(out=mlpw_t[:], in_=t_mlpw[:])
            mlpb_t = cp.tile([128, 2], f32)
            nc.sync.dma_start(out=mlpb_t[:], in_=t_mlpb[:])
            wih_t = cp.tile([128, 2 * G3], bf16)
            nc.sync.dma_start(out=wih_t[:], in_=t_wih[:])
            whh_t = cp.tile([D, G3], bf16)
            nc.sync.dma_start(out=whh_t[:], in_=t_whh[:])
            br_t = cp.tile([D, 1], f32)
            nc.sync.dma_start(out=br_t[:], in_=t_br[:])
            bz_t = cp.tile([D, 1], f32)
            nc.sync.dma_start(out=bz_t[:], in_=t_bz[:])
            bin_t = cp.tile([D, 1], f32)
            nc.sync.dma_start(out=bin_t[:], in_=t_bin[:])
            bhn_t = cp.tile([D, 1], f32)
            nc.sync.dma_start(out=bhn_t[:], in_=t_bhn[:])
            ident_t = cp.tile([128, 128], f32)
            nc.sync.dma_start(out=ident_t[:], in_=t_ident[:])

            with tc.tile_pool(name="mm", bufs=2) as mp, \
                 tc.tile_pool(name="pp", bufs=1, space="PSUM") as pp2, \
                 tc.tile_pool(name="pt", bufs=2, space="PSUM") as ppt:
                for it in range(n_nt):
                    lo = it * NT
                    hi = min(lo + NT, nloc)
                    n = hi - lo
                    xb = mp.tile([D, NT], bf16, tag="xb")
                    nc.sync.dma_start(out=xb[:, :n], in_=t_xtb[:, lo:hi])
                    xf = mp.tile([D, NT], f32, tag="xf")
                    nc.sync.dma_start(out=xf[:, :n], in_=t_xtf[:, lo:hi])
                    hid = []
                    for k in range(2):
                        hk = mp.tile([128, NT], bf16, tag=f"hid{k}")
                        hid.append(hk)
                    # ---- MLP: hidden[k] = relu(sum_t Wt @ upd_t + b)
                    for k in range(2):
                        ph = pp2.tile([128, NT], f32, tag="ph")
                        for t in range(T_TYPES):
                            nc.tensor.matmul(
                                out=ph[:, :n],
                                lhsT=mlpw_t[:, (k * 4 + t) * 128:(k * 4 + t + 1) * 128],
                                rhs=upd[:, t * nloc + lo:t * nloc + hi],
                                start=(t == 0), stop=(t == 3),
                            )
                        nc.scalar.activation(
                            hid[k][:, :n], ph[:, :n],
                            mybir.ActivationFunctionType.Relu,
                            bias=mlpb_t[:, k:k + 1], scale=1.0,
                        )
                    # ---- GRU gates
                    # r and z gates, each [64, n] on partitions 0:63
                    gate_sb = []
                    for gi_, bias_t in ((0, br_t), (1, bz_t)):
                        pg = pp2.tile([D, NT], f32, tag=f"pg{gi_}")
                        for hc in range(2):
                            nc.tensor.matmul(
                                out=pg[:, :n],
                                lhsT=wih_t[:, hc * G3 + gi_ * D:hc * G3 + (gi_ + 1) * D],
                                rhs=hid[hc][:, :n],
                                start=(hc == 0), stop=False,
                            )
                        nc.tensor.matmul(
                            out=pg[:, :n], lhsT=whh_t[:, gi_ * D:(gi_ + 1) * D],
                            rhs=xb[:, :n], start=False, stop=True,
                        )
                        gsb = mp.tile([D, NT], f32, tag=f"g{gi_}")
                        nc.scalar.activation(
                            gsb[:, :n], pg[:, :n],
                            mybir.ActivationFunctionType.Sigmoid,
                            bias=bias_t[:], scale=1.0,
                        )
                        gate_sb.append(gsb)
                    r_sb, z_sb = gate_sb
                    # i_n psum [64, n]
                    pin = pp2.tile([D, NT], f32, tag="pin")
                    for hc in range(2):
                        nc.tensor.matmul(
                            out=pin[:, :n],
                            lhsT=wih_t[:, hc * G3 + 128:hc * G3 + G3],
                            rhs=hid[hc][:, :n],
                            start=(hc == 0), stop=(hc == 1),
                        )
                    # h_n psum [64, n]
                    phn = pp2.tile([D, NT], f32, tag="phn")
                    nc.tensor.matmul(
                        out=phn[:, :n], lhsT=whh_t[:, 128:G3],
                        rhs=xb[:, :n], start=True, stop=True,
                    )
                    hn = mp.tile([D, NT], f32, tag="hn")
                    nc.scalar.activation(
                        hn[:, :n], phn[:, :n],
                        mybir.ActivationFunctionType.Identity,
                        bias=bhn_t[:], scale=1.0,
                    )
                    t1 = mp.tile([D, NT], f32, tag="t1")
                    nc.vector.tensor_mul(t1[:, :n], r_sb[:, :n], hn[:, :n])
                    # t2 = (pin + b_in) + t1
                    t2 = mp.tile([D, NT], f32, tag="t2")
                    nc.vector.scalar_tensor_tensor(
                        t2[:, :n], pin[:, :n], bin_t[:], t1[:, :n],
                        mybir.AluOpType.add, mybir.AluOpType.add,
                    )
                    ng = mp.tile([D, NT], f32, tag="ng")
                    nc.scalar.activation(
                        ng[:, :n], t2[:, :n],
                        mybir.ActivationFunctionType.Tanh,
                        bias=0.0, scale=1.0,
                    )
                    # out = n + z*(x - n)
                    t3 = mp.tile([D, NT], f32, tag="t3")
                    nc.vector.tensor_sub(t3[:, :n], xf[:, :n], ng[:, :n])
                    t4 = mp.tile([D, NT], f32, tag="t4")
                    nc.vector.tensor_mul(t4[:, :n], z_sb[:, :n], t3[:, :n])
                    ot = mp.tile([D, NT], f32, tag="ot")
                    nc.vector.tensor_add(ot[:, :n], ng[:, :n], t4[:, :n])
                    # ---- transpose to rows and store
                    for q in range(0, NT, 128):
                        if lo + q >= nloc:
                            break
                        qn = min(128, nloc - lo - q)
                        pt = ppt.tile([128, D], f32, tag="pt")
                        nc.tensor.transpose(
                            out=pt[:], in_=ot[:, q:q + 128],
                            identity=ident_t[0:D, 0:D],
                        )
                        rows = mp.tile([128, D], f32, tag="rows")
                        nc.vector.tensor_copy(rows[:], pt[:])
                        nc.sync.dma_start(
                            out=t_out[lo + q:lo + q + qn, :],
                            in_=rows[:qn, :])

    nc.compile()
    return nc


# ---------------------------------------------------------------- entry

_CACHE = {}


def _build_in_maps(inputs, dims):
    node_feature = np.asarray(inputs["node_feature"], np.float32)
    per_core_arrays, tbl, meta = _host_prep(
        node_feature, np.asarray(inputs["edge_index"]),
        np.asarray(inputs["edge_type"]),
        np.asarray(inputs["edge_weight"], np.float32), dims)
    wts = _prep_weights(
        np.asarray(inputs["mlp_W"], np.float32),
        np.asarray(inputs["mlp_b"], np.float32),
        np.asarray(inputs["w_ih"], np.float32),
        np.asarray(inputs["w_hh"], np.float32),
        np.asarray(inputs["b_ih"], np.float32),
        np.asarray(inputs["b_hh"], np.float32), dims["nloc"])

    nloc = dims["nloc"]
    ntp = (nloc + 127) // 128 * 128
    in_maps = []
    for c in range(N_CORES):
        x_own = node_feature[c * nloc:(c + 1) * nloc]       # [nloc, 64]
        xt = np.zeros((D, ntp), np.float32)
        xt[:, :nloc] = x_own.T
        m = dict(per_core_arrays[c])
        m.update(
            tbl=tbl,
            xtb=xt.astype(BF16), xtf=xt,
            mlpw=wts["mlpw"], mlpb=wts["mlpb"], wih=wts["wih"],
            whh=wts["whh"], br=wts["b_r"], bz=wts["b_z"], bin=wts["b_in"],
            bhn=wts["b_hn"], ident=wts["ident"],
        )
        in_maps.append(m)
    return in_maps, meta


def _run(inputs, trace=False):
    _register_ntff_hook()
    dims = dict(REAL)
    in_maps, meta = _build_in_maps(inputs, dims)
    key = ("real", tuple(meta["ncs"]))
    if key not in _CACHE:
        _CACHE[key] = _build_program(dims, meta)
    nc = _CACHE[key]
    res = run_bass_kernel_spmd(nc, in_maps, list(range(N_CORES)), trace=trace)
    nloc = dims["nloc"]
    out = np.concatenate(
        [res.results[c]["out"][:nloc] for c in range(N_CORES)], axis=0)
    return out.astype(np.float32), res


def kernel(**inputs) -> np.ndarray:
    return _run(inputs, trace=False)[0]
